# revision 13
# baseline (speedup 1.0000x reference)
"""Trainium2 Bass kernel for nn_AlignCriterion (align loss).

Data-parallel over batch: 8 batches per core, 8 cores. The O(B*N^2*C)
correlation/assignment einsums are algebraically collapsed (see _combine).

Layouts shipped from host per batch:
  natural  [128, 7, 385] bf16   x with a ones column  (P/R moving operand)
  transposed [128, 3, 896] fp8  x^T, n padded to 896  (asg moving operand)
  ztq      [128, 3, 16]   fp8   8 * normalized queries^T (asg stationary)
  ztb      [128, 3, 16]   bf16  raw queries^T (CE gram matrix)
  misc     [128, 7, 2, 3] f32   per-row 1/||x||: [inv, inv/8, invR]
  u        [128, 7, 2]    f32   attention masks (t, side)

Device per batch: sim = z z^T; asgT = ztq^T @ xT (both sides into one
PSUM tile, lc at rows 32:37 via tile_position); relu-drain to bf16;
7 combined PE transposes -> [128, 7, (2,5)]; masked softmax weights
wt = [wg*inv | invR | wg] (11 cols/side); P/R matmuls (R at rows 32:43).
The ones column gives beta/v; the inv column gives s_gc / s_lc/784.
Host combines partials in f64. Emission is software-pipelined: batch
b's transposes/PR are emitted after batch b+1's asg matmuls so the PE
stream never stalls on the DVE/ACT softmax round-trip.
"""

import sys

import numpy as np

sys.path.insert(0, "/opt/trn_rl_repo")

import ml_dtypes  # noqa: E402
import concourse.bass as bass  # noqa: E402,F401
import concourse.mybir as mybir  # noqa: E402
import concourse.tile as tile  # noqa: E402
from concourse import bacc  # noqa: E402
from concourse.bass_utils import run_bass_kernel_spmd  # noqa: E402
from concourse.masks import make_identity  # noqa: E402

F32 = mybir.dt.float32
BF16 = mybir.dt.bfloat16
FP8 = mybir.dt.float8e4
AF = mybir.ActivationFunctionType
ALU = mybir.AluOpType
AX = mybir.AxisListType

BF = ml_dtypes.bfloat16
F8 = ml_dtypes.float8_e4m3

B = 64
N = 784          # 28*28 spatial positions
C = 384
Q = 5
NCORES = 8
BL = B // NCORES  # batches per core = 8
NT = 7           # n tiles of 128
NK = 3           # c chunks of 128
NP = 896         # padded n for the transposed layout (7*128)
H = NP // 2      # psum half width (448)

_CACHED = {}


def _build():
    nc = bacc.Bacc("TRN2", target_bir_lowering=False, debug=False,
                   num_devices=NCORES)

    natg = nc.dram_tensor("natg", [BL, 128, NT, C + 1], BF16, kind="ExternalInput").ap()
    natl = nc.dram_tensor("natl", [BL, 128, NT, C + 1], BF16, kind="ExternalInput").ap()
    trag = nc.dram_tensor("trag", [BL, 128, NK, NP], FP8, kind="ExternalInput").ap()
    tral = nc.dram_tensor("tral", [BL, 128, NK, NP], FP8, kind="ExternalInput").ap()
    ztq = nc.dram_tensor("ztq", [BL, 128, NK, 16], FP8, kind="ExternalInput").ap()
    ztb = nc.dram_tensor("ztb", [BL, 128, NK, 16], BF16, kind="ExternalInput").ap()
    misc = nc.dram_tensor("misc", [128, BL, NT, 2, 3], F32, kind="ExternalInput").ap()
    s_in = nc.dram_tensor("s_in", [48, 16], BF16, kind="ExternalInput").ap()
    u_in = nc.dram_tensor("u_in", [128, BL, NT, 2, 2], F32, kind="ExternalInput").ap()
    out = nc.dram_tensor("out", [BL, 48, 400], F32, kind="ExternalOutput").ap()

    with tile.TileContext(nc) as tc:
        _kernel(tc, out, natg, natl, trag, tral, ztq, ztb, misc, u_in, s_in)

    # the installed walrus birverifier rejects EVENT_SEMAPHORE_RANGE_CLEAR
    # (opcode 176, emitted by the Tile kernel-tail sem cleanup). NRT re-inits
    # semaphores per execution, so drop the tail clear entirely.
    for fn in nc.m.functions:
        for blk in fn.blocks:
            il = blk.instructions
            for i in range(len(il) - 1, -1, -1):
                if isinstance(il[i], mybir.InstISA) and il[i].isa_opcode == 176:
                    del il[i]

    nc.compile()
    return nc


def _kernel(tc, out, natg, natl, trag, tral, ztq, ztb, misc, u_in, s_in):
    from contextlib import ExitStack
    with ExitStack() as ctx:
        _kernel_inner(ctx, tc, out, natg, natl, trag, tral, ztq, ztb, misc,
                      u_in, s_in)


def _kernel_inner(ctx, tc, out, natg, natl, trag, tral, ztq, ztb, misc,
                  u_in, s_in):
    nc = tc.nc

    consts = ctx.enter_context(tc.tile_pool(name="consts", bufs=1))
    sbin = ctx.enter_context(tc.tile_pool(name="sbin", bufs=4))
    sbq = ctx.enter_context(tc.tile_pool(name="sbq", bufs=3))
    sbs = ctx.enter_context(tc.tile_pool(name="sbs", bufs=3))
    sbo = ctx.enter_context(tc.tile_pool(name="sbo", bufs=4))
    ps_aa = ctx.enter_context(tc.tile_pool(name="ps_aa", bufs=2, space="PSUM"))
    ps_tp = ctx.enter_context(tc.tile_pool(name="ps_tp", bufs=1, space="PSUM"))
    ps_pr = ctx.enter_context(tc.tile_pool(name="ps_pr", bufs=3, space="PSUM"))

    S = consts.tile([48, 16], BF16, tag="S")
    nc.sync.dma_start(S[:], s_in[:, :])
    U = consts.tile([128, BL, NT, 2, 2], F32, tag="U")
    nc.sync.dma_start(U[:], u_in[:, :, :, :, :])
    MI = consts.tile([128, BL, NT, 2, 3], F32, tag="MI")
    nc.sync.dma_start(MI[:], misc[:, :, :, :, :])

    asgT_slots = []
    for j in range(3):
        t = consts.tile([48, NP], BF16, tag=f"asgT{j}", name=f"asgT{j}")
        nc.gpsimd.memset(t[:], 0.0)
        asgT_slots.append(t)
    out_slots = []
    for j in range(4):
        t = consts.tile([48, 400], F32, tag=f"outsb{j}", name=f"outsb{j}")
        nc.gpsimd.memset(t[:], 0.0)
        out_slots.append(t)

    st = [None] * BL  # per-batch live tiles for the lagged stage

    def stage_a(b):
        ng = sbin.tile([128, NT, C + 1], BF16, tag="ng")
        nl = sbin.tile([128, NT, C + 1], BF16, tag="nl")
        tg = sbin.tile([128, NK, NP], FP8, tag="tg")
        tl = sbin.tile([128, NK, NP], FP8, tag="tl")
        nc.sync.dma_start(ng[:], natg[b])
        nc.sync.dma_start(nl[:], natl[b])
        nc.sync.dma_start(tg[:], trag[b])
        nc.sync.dma_start(tl[:], tral[b])
        zq = sbq.tile([128, NK, 16], FP8, tag="zq")
        zb = sbq.tile([128, NK, 16], BF16, tag="zb")
        nc.sync.dma_start(zq[:], ztq[b])
        nc.sync.dma_start(zb[:], ztb[b])

        out_sb = out_slots[b % 4]
        asgT = asgT_slots[b % 3]

        # sim (CE gram) + assignment logits share one 2-bank psum tile
        aa_ps = ps_aa.tile([48, 2, 512], F32, tag="aa_ps")
        for k in range(NK):
            nc.tensor.matmul(aa_ps[0:10, 0, 448:458], zb[:, k, 0:10],
                             zb[:, k, 0:10],
                             start=(k == 0), stop=(k == NK - 1))
        for h in range(2):
            for k in range(NK):
                nc.tensor.matmul(aa_ps[0:5, h, 0:H], zq[:, k, 0:Q],
                                 tg[:, k, H * h:H * (h + 1)],
                                 start=(k == 0), stop=(k == NK - 1))
        for h in range(2):
            for k in range(NK):
                nc.tensor.matmul(aa_ps[32:37, h, 0:H], zq[:, k, Q:10],
                                 tl[:, k, H * h:H * (h + 1)],
                                 start=(k == 0), stop=(k == NK - 1),
                                 tile_position=(0, 32))

        # relu + drain to bf16 (both on DVE, back to back: one queue, no
        # cross-engine WAW stall on the shared asgT tile)
        nc.vector.tensor_scalar_max(
            asgT[0:5, 0:NP].rearrange("p (h n) -> p h n", h=2),
            aa_ps[0:5, :, 0:H], 0.0)
        nc.vector.tensor_scalar_max(
            asgT[32:37, 0:NP].rearrange("p (h n) -> p h n", h=2),
            aa_ps[32:37, :, 0:H], 0.0)
        nc.scalar.copy(out_sb[0:10, 385:395], aa_ps[0:10, 0, 448:458])
        st[b] = (ng, nl, asgT, out_sb)

    def stage_tp(b):
        ng, nl, asgT, out_sb = st[b]

        # selector-"transpose" rows {0:5, 32:37} -> [128, 7, 10]:
        # plain matmul out = asgT.T @ S (stationary = asgT chunk)
        tp_ps = ps_tp.tile([128, NT, 10], F32, tag="tp_ps")
        for t in range(NT):
            nc.tensor.matmul(tp_ps[:, t, :],
                             asgT[0:37, 128 * t:128 * (t + 1)],
                             S[0:37, 0:10], start=True, stop=True)

        inv_ts = MI[:, b, :, :, 0]
        inv8_ts = MI[:, b, :, :, 1]
        invR_ts = MI[:, b, :, :, 2]

        # e = exp(asg * inv/8) ; tp cols: gc 0:5, lc 5:10
        e_in = sbs.tile([128, NT, 10], BF16, tag="e_in")
        nc.vector.tensor_tensor(
            out=e_in[:].rearrange("p t (s q) -> p t s q", s=2),
            in0=tp_ps[:].rearrange("p t (s q) -> p t s q", s=2),
            in1=inv8_ts.broadcast_to([128, NT, 2, Q]), op=ALU.mult)
        e = sbs.tile([128, NT, 10], BF16, tag="e")
        nc.scalar.activation(e[:], e_in[:], AF.Exp)

        sume = sbs.tile([128, NT, 2], F32, tag="sume")
        nc.vector.tensor_reduce(
            sume[:], e[:].rearrange("p t (s q) -> p t s q", s=2),
            axis=AX.X, op=ALU.add)
        sumr = sbs.tile([128, NT, 2], F32, tag="sumr")
        nc.vector.reciprocal(sumr[:], sume[:])
        # stil0 = mask/sume ; stil = (mask*inv)/sume  (U*inv shipped)
        stil0 = sbs.tile([128, NT, 2], F32, tag="stil0")
        nc.vector.tensor_tensor(out=stil0[:], in0=sumr[:], in1=U[:, b, :, :, 0],
                                op=ALU.mult)
        stil = sbs.tile([128, NT, 2], F32, tag="stil")
        nc.vector.tensor_tensor(out=stil[:], in0=sumr[:], in1=U[:, b, :, :, 1],
                                op=ALU.mult)

        # wt columns per side: [wg*inv x5 | invR | wg x5 | pad]
        wt = sbs.tile([128, NT, 24], BF16, tag="wt")
        wt4 = wt[:].rearrange("p t (s c) -> p t s c", s=2)
        e4 = e[:].rearrange("p t (s q) -> p t s q", s=2)
        nc.vector.tensor_tensor(out=wt4[:, :, :, 0:Q], in0=e4[:],
                                in1=stil[:].broadcast_to([128, NT, 2, Q]),
                                op=ALU.mult)
        nc.vector.tensor_tensor(out=wt4[:, :, :, 6:6 + Q], in0=e4[:],
                                in1=stil0[:].broadcast_to([128, NT, 2, Q]),
                                op=ALU.mult)
        nc.vector.tensor_copy(wt4[:, :, :, Q:6],
                              invR_ts.broadcast_to([128, NT, 2, 1]))
        st[b] = (ng, nl, wt, out_sb)

    def stage_pr(b):
        ng, nl, wt, out_sb = st[b]
        st[b] = None

        # P rows 0:11, R rows 32:43
        pr_ps = ps_pr.tile([48, C + 1], F32, tag="pr_ps")
        for t in range(NT):
            nc.tensor.matmul(pr_ps[0:11, :], wt[:, t, 0:11], ng[:, t, :],
                             start=(t == 0), stop=(t == NT - 1))
        for t in range(NT):
            nc.tensor.matmul(pr_ps[32:43, :], wt[:, t, 12:23], nl[:, t, :],
                             start=(t == 0), stop=(t == NT - 1),
                             tile_position=(0, 32))
        nc.scalar.copy(out_sb[0:11, 0:C + 1], pr_ps[0:11, :])
        nc.scalar.copy(out_sb[32:43, 0:C + 1], pr_ps[32:43, :])
        nc.gpsimd.dma_start(out[b], out_sb[:])

    for i in range(BL + 2):
        if i < BL:
            stage_a(i)
        if 1 <= i <= BL:
            stage_tp(i - 1)
        if i >= 2:
            stage_pr(i - 2)


def _neg_index():
    n2 = 2 * Q
    mask = np.ones((n2, n2), dtype=bool)
    np.fill_diagonal(mask, False)
    for i in range(Q):
        mask[i, Q + i] = False
        mask[Q + i, i] = False
    return np.stack([np.where(mask[r])[0] for r in range(n2)])


def _combine(results):
    T1 = 0.0
    G = 0.0
    alphas = []
    betas = []
    vs = []
    sims = []
    for r in results:
        o = np.asarray(r["out"], dtype=np.float64)  # [BL, 48, 400]
        P = o[:, 0:11, 0:C + 1]
        R = o[:, 32:43, 0:C + 1]
        sims.append(o[:, 0:10, 385:395])
        Pq, beta, sgc = P[:, 0:Q, 0:C], P[:, 6:6 + Q, C], P[:, Q, 0:C]
        Rq, v, slc = R[:, 0:Q, 0:C], R[:, 6:6 + Q, C], R[:, Q, 0:C]
        T1 += (Pq * Rq).sum()
        G += (sgc * slc).sum()
        alphas.append(np.einsum("bqc,bc->bq", Pq, slc))
        betas.append(beta)
        vs.append(v)
    alpha = np.concatenate(alphas, 0)
    beta = np.concatenate(betas, 0)
    v = np.concatenate(vs, 0)
    g = G / (B * N)
    T2 = ((alpha + (0.1 - g) * beta) * v).sum()
    loss1 = -0.15 * (T1 - T2)

    # query CE from raw gram matrices
    sim = np.concatenate(sims, 0)  # [B, 10, 10]
    d = np.einsum("bii->bi", sim)
    iq = 1.0 / np.maximum(np.sqrt(d), 1e-10)
    sh = sim * iq[:, :, None] * iq[:, None, :]
    rows = np.arange(2 * Q)
    pos = sh[:, rows, (rows + Q) % (2 * Q)]          # [B, 10]
    negs = sh[:, rows[:, None], _NEG_IDX]            # [B, 10, 8]
    logits = np.concatenate([pos[:, :, None], negs], axis=-1)
    m = logits.max(-1)
    ce = m + np.log(np.exp(logits - m[:, :, None]).sum(-1)) - pos
    loss2 = ce.mean()
    return np.float32(loss1 + loss2)


_NEG_IDX = _neg_index()


def _prep(gc, lc, q0, q1, att):
    """Build per-core input maps (host-side sharding + layout)."""
    # natural bf16 with ones column: [B, 128, 7, 385]
    def nat_pack(x):
        pad = np.zeros((B, NT * 128, C + 1), BF)
        pad[:, :N, :C] = x.astype(BF)
        pad[:, :, C] = 1.0
        return np.ascontiguousarray(
            pad.reshape(B, NT, 128, C + 1).transpose(0, 2, 1, 3))

    # transposed fp8: [B, 128, 3, 896]
    def tra_pack(x):
        t8 = np.zeros((B, C, NP), F8)
        t8[:, :, :N] = np.swapaxes(x, 1, 2).astype(F8)
        return np.ascontiguousarray(t8.reshape(B, NK, 128, NP).transpose(0, 2, 1, 3))

    z = np.concatenate([q0, q1], axis=1)  # [B, 10, 384]
    qn = np.linalg.norm(z, axis=-1)       # [B, 10]
    zhat8 = 8.0 * z / np.maximum(qn, 1e-10)[:, :, None]
    def z_pack(zv, dt):
        zt = np.zeros((B, C, 16), np.float32)
        zt[:, :, 0:10] = np.swapaxes(zv, 1, 2)
        return np.ascontiguousarray(
            zt.reshape(B, NK, 128, 16).transpose(0, 2, 1, 3).astype(dt))
    ztq_a = z_pack(zhat8, F8)
    ztb_a = z_pack(z, BF)

    natg_a = nat_pack(gc)
    natl_a = nat_pack(lc)
    trag_a = tra_pack(gc)
    tral_a = tra_pack(lc)

    # row inverse norms on host: inv [2, B, 7, 128] (s: 0=gc, 1=lc)
    nrm = np.stack([np.linalg.norm(gc, axis=-1), np.linalg.norm(lc, axis=-1)])
    invf = np.zeros((2, B, NT * 128), np.float32)
    invf[:, :, :N] = 1.0 / np.maximum(nrm, 1e-10)
    invf[:, :, N:] = 1e10
    invf = invf.reshape(2, B, NT, 128)
    mi = np.zeros((128, B, NT, 2, 3), np.float32)
    mi[:, :, :, :, 0] = invf.transpose(3, 1, 2, 0)
    mi[:, :, :, :, 1] = mi[:, :, :, :, 0] / 8.0
    mi[:, :, :, 0, 2] = mi[:, :, :, 0, 0]
    mi[:, :, :, 1, 2] = mi[:, :, :, 1, 0] / N

    S_sel = np.zeros((48, 16), BF)
    S_sel[np.arange(5), np.arange(5)] = 1
    S_sel[np.arange(32, 37), np.arange(5, 10)] = 1

    # mask U: [128, B, 7, 2]
    af = att.astype(np.float32)  # [128, 784]
    Uf = np.zeros((2, B, NT, 128), np.float32)
    Uf[0, :, :6, :] = af[:B, :768].reshape(B, 6, 128)
    Uf[0, :, 6, :16] = af[:B, 768:]
    Uf[1, :, :6, :] = af[B:, :768].reshape(B, 6, 128)
    Uf[1, :, 6, :16] = af[B:, 768:]
    Uf = Uf.transpose(3, 1, 2, 0)  # [128, B, 7, 2]
    U2 = np.stack([Uf, Uf * mi[:, :, :, :, 0]], axis=-1)  # [128, B, 7, 2, 2]

    in_maps = []
    for i in range(NCORES):
        s = slice(i * BL, (i + 1) * BL)
        in_maps.append({
            "natg": natg_a[s], "natl": natl_a[s],
            "trag": trag_a[s], "tral": tral_a[s],
            "ztq": ztq_a[s], "ztb": ztb_a[s],
            "misc": np.ascontiguousarray(mi[:, s]),
            "s_in": S_sel,
            "u_in": np.ascontiguousarray(U2[:, s]),
        })
    return in_maps


def kernel(all_queries_0, all_queries_1, gc_output, lc_output, attn_hard,
           gc_spatial_res=None, lc_spatial_res=None):
    if "nc" not in _CACHED:
        _CACHED["nc"] = _build()
    nc = _CACHED["nc"]

    gc = np.asarray(gc_output, dtype=np.float32)
    lc = np.asarray(lc_output, dtype=np.float32)[:, 0]
    q0 = np.asarray(all_queries_0, dtype=np.float32)
    q1 = np.asarray(all_queries_1, dtype=np.float32)
    att = np.asarray(attn_hard, dtype=np.int32).reshape(2 * B, N)

    in_maps = _prep(gc, lc, q0, q1, att)
    res = run_bass_kernel_spmd(nc, in_maps, core_ids=list(range(NCORES)))
    return _combine(res.results)


# revision 15
# speedup vs baseline: 1.3615x; 1.3615x over previous
"""Trainium2 Bass kernel for nn_AlignCriterion (align loss).

Data-parallel over batch: 8 batches per core, 8 cores. The O(B*N^2*C)
correlation/assignment einsums are algebraically collapsed (see _combine).

Layouts shipped from host per batch:
  natural  [128, 7, 385] bf16   x with a ones column  (P/R moving operand)
  transposed [128, 3, 896] fp8  x^T, n padded to 896  (asg moving operand)
  ztq      [128, 3, 16]   fp8   8 * normalized queries^T (asg stationary)
  ztb      [128, 3, 16]   bf16  raw queries^T (CE gram matrix)
  misc     [128, 7, 2, 3] f32   per-row 1/||x||: [inv, inv/8, invR]
  u        [128, 7, 2]    f32   attention masks (t, side)

Device per batch: sim = z z^T; asgT = ztq^T @ xT (both sides into one
PSUM tile, lc at rows 32:37 via tile_position); relu-drain to bf16;
7 combined PE transposes -> [128, 7, (2,5)]; masked softmax weights
wt = [wg*inv | invR | wg] (11 cols/side); P/R matmuls (R at rows 32:43).
The ones column gives beta/v; the inv column gives s_gc / s_lc/784.
Host combines partials in f64. Emission is software-pipelined: batch
b's transposes/PR are emitted after batch b+1's asg matmuls so the PE
stream never stalls on the DVE/ACT softmax round-trip.
"""

import sys

import numpy as np

sys.path.insert(0, "/opt/trn_rl_repo")

import ml_dtypes  # noqa: E402
import concourse.bass as bass  # noqa: E402,F401
import concourse.mybir as mybir  # noqa: E402
import concourse.tile as tile  # noqa: E402
from concourse import bacc  # noqa: E402
from concourse.bass_utils import run_bass_kernel_spmd  # noqa: E402
from concourse.masks import make_identity  # noqa: E402

F32 = mybir.dt.float32
BF16 = mybir.dt.bfloat16
FP8 = mybir.dt.float8e4
AF = mybir.ActivationFunctionType
ALU = mybir.AluOpType
AX = mybir.AxisListType

BF = ml_dtypes.bfloat16
F8 = ml_dtypes.float8_e4m3

B = 64
N = 784          # 28*28 spatial positions
C = 384
Q = 5
NCORES = 8
BL = B // NCORES  # batches per core = 8
NT = 7           # n tiles of 128
NK = 3           # c chunks of 128
NP = 896         # padded n for the transposed layout (7*128)
H = NP // 2      # psum half width (448)

_CACHED = {}


def _build():
    nc = bacc.Bacc("TRN2", target_bir_lowering=False, debug=False,
                   num_devices=NCORES)

    natg = nc.dram_tensor("natg", [BL, 128, NT, C + 1], FP8, kind="ExternalInput").ap()
    natl = nc.dram_tensor("natl", [BL, 128, NT, C + 1], FP8, kind="ExternalInput").ap()
    trag = nc.dram_tensor("trag", [BL, 128, NK, NP], FP8, kind="ExternalInput").ap()
    tral = nc.dram_tensor("tral", [BL, 128, NK, NP], FP8, kind="ExternalInput").ap()
    ztq = nc.dram_tensor("ztq", [BL, 128, NK, 16], FP8, kind="ExternalInput").ap()
    ztb = nc.dram_tensor("ztb", [BL, 128, NK, 16], BF16, kind="ExternalInput").ap()
    misc = nc.dram_tensor("misc", [128, BL, NT, 2, 3], F32, kind="ExternalInput").ap()
    s_in = nc.dram_tensor("s_in", [48, 16], BF16, kind="ExternalInput").ap()
    u_in = nc.dram_tensor("u_in", [128, BL, NT, 2, 2], F32, kind="ExternalInput").ap()
    out = nc.dram_tensor("out", [BL, 48, 400], F32, kind="ExternalOutput").ap()

    with tile.TileContext(nc) as tc:
        _kernel(tc, out, natg, natl, trag, tral, ztq, ztb, misc, u_in, s_in)

    # the installed walrus birverifier rejects EVENT_SEMAPHORE_RANGE_CLEAR
    # (opcode 176, emitted by the Tile kernel-tail sem cleanup). NRT re-inits
    # semaphores per execution, so drop the tail clear entirely.
    for fn in nc.m.functions:
        for blk in fn.blocks:
            il = blk.instructions
            for i in range(len(il) - 1, -1, -1):
                if isinstance(il[i], mybir.InstISA) and il[i].isa_opcode == 176:
                    del il[i]

    nc.compile()
    return nc


def _kernel(tc, out, natg, natl, trag, tral, ztq, ztb, misc, u_in, s_in):
    from contextlib import ExitStack
    with ExitStack() as ctx:
        _kernel_inner(ctx, tc, out, natg, natl, trag, tral, ztq, ztb, misc,
                      u_in, s_in)


def _kernel_inner(ctx, tc, out, natg, natl, trag, tral, ztq, ztb, misc,
                  u_in, s_in):
    nc = tc.nc

    consts = ctx.enter_context(tc.tile_pool(name="consts", bufs=1))
    sbin = ctx.enter_context(tc.tile_pool(name="sbin", bufs=3))
    sbnat = ctx.enter_context(tc.tile_pool(name="sbnat", bufs=8))
    sbq = ctx.enter_context(tc.tile_pool(name="sbq", bufs=3))
    sbs = ctx.enter_context(tc.tile_pool(name="sbs", bufs=4))
    sbo = ctx.enter_context(tc.tile_pool(name="sbo", bufs=4))
    ps_aa = ctx.enter_context(tc.tile_pool(name="ps_aa", bufs=2, space="PSUM"))
    ps_tp = ctx.enter_context(tc.tile_pool(name="ps_tp", bufs=2, space="PSUM"))
    ps_pr = ctx.enter_context(tc.tile_pool(name="ps_pr", bufs=2, space="PSUM"))

    S = consts.tile([48, 16], BF16, tag="S")
    nc.sync.dma_start(S[:], s_in[:, :])
    U = consts.tile([128, BL, NT, 2, 2], F32, tag="U")
    nc.sync.dma_start(U[:], u_in[:, :, :, :, :])
    MI = consts.tile([128, BL, NT, 2, 3], F32, tag="MI")
    nc.sync.dma_start(MI[:], misc[:, :, :, :, :])

    asgT_slots = []
    for j in range(BL):
        t = consts.tile([48, NP], BF16, tag=f"asgT{j}", name=f"asgT{j}")
        nc.gpsimd.memset(t[:], 0.0)
        asgT_slots.append(t)
    out_slots = []
    for j in range(BL):
        t = consts.tile([48, 400], F32, tag=f"outsb{j}", name=f"outsb{j}")
        nc.gpsimd.memset(t[:], 0.0)
        out_slots.append(t)

    st = [None] * BL  # per-batch live tiles for the lagged stage

    def stage_a(b):
        ng = sbnat.tile([128, NT, C + 1], FP8, tag="ng")
        nl = sbnat.tile([128, NT, C + 1], FP8, tag="nl")
        tg = sbin.tile([128, NK, NP], FP8, tag="tg")
        tl = sbin.tile([128, NK, NP], FP8, tag="tl")
        nc.sync.dma_start(ng[:], natg[b])
        nc.sync.dma_start(nl[:], natl[b])
        nc.sync.dma_start(tg[:], trag[b])
        nc.sync.dma_start(tl[:], tral[b])
        zq = sbq.tile([128, NK, 16], FP8, tag="zq")
        zb = sbq.tile([128, NK, 16], BF16, tag="zb")
        nc.sync.dma_start(zq[:], ztq[b])
        nc.sync.dma_start(zb[:], ztb[b])

        out_sb = out_slots[b]
        asgT = asgT_slots[b]

        # sim (CE gram) + assignment logits share one 2-bank psum tile
        aa_ps = ps_aa.tile([48, 2, 512], F32, tag="aa_ps")
        for k in range(NK):
            nc.tensor.matmul(aa_ps[0:10, 0, 448:458], zb[:, k, 0:10],
                             zb[:, k, 0:10],
                             start=(k == 0), stop=(k == NK - 1))
        for h in range(2):
            for k in range(NK):
                nc.tensor.matmul(aa_ps[0:5, h, 0:H], zq[:, k, 0:Q],
                                 tg[:, k, H * h:H * (h + 1)],
                                 start=(k == 0), stop=(k == NK - 1))
        for h in range(2):
            for k in range(NK):
                nc.tensor.matmul(aa_ps[32:37, h, 0:H], zq[:, k, Q:10],
                                 tl[:, k, H * h:H * (h + 1)],
                                 start=(k == 0), stop=(k == NK - 1),
                                 tile_position=(0, 32))

        # relu + drain to bf16 (both on DVE, back to back: one queue, no
        # cross-engine WAW stall on the shared asgT tile)
        nc.vector.tensor_scalar_max(
            asgT[0:5, 0:NP].rearrange("p (h n) -> p h n", h=2),
            aa_ps[0:5, :, 0:H], 0.0)
        nc.vector.tensor_scalar_max(
            asgT[32:37, 0:NP].rearrange("p (h n) -> p h n", h=2),
            aa_ps[32:37, :, 0:H], 0.0)
        nc.scalar.copy(out_sb[0:10, 385:395], aa_ps[0:10, 0, 448:458])
        st[b] = (ng, nl, asgT, out_sb)

    def stage_tp(b):
        ng, nl, asgT, out_sb = st[b]

        # selector-"transpose" rows {0:5, 32:37} -> [128, 7, 10]:
        # plain matmul out = asgT.T @ S (stationary = asgT chunk)
        tp_ps = ps_tp.tile([128, NT, 10], F32, tag="tp_ps")
        for t in range(NT):
            nc.tensor.matmul(tp_ps[:, t, :],
                             asgT[0:37, 128 * t:128 * (t + 1)],
                             S[0:37, 0:10], start=True, stop=True)

        inv_ts = MI[:, b, :, :, 0]
        inv8_ts = MI[:, b, :, :, 1]
        invR_ts = MI[:, b, :, :, 2]

        # e = exp(asg * inv/8) ; tp cols: gc 0:5, lc 5:10
        e_in = sbs.tile([128, NT, 10], BF16, tag="e_in")
        nc.vector.tensor_tensor(
            out=e_in[:].rearrange("p t (s q) -> p t s q", s=2),
            in0=tp_ps[:].rearrange("p t (s q) -> p t s q", s=2),
            in1=inv8_ts.broadcast_to([128, NT, 2, Q]), op=ALU.mult)
        e = sbs.tile([128, NT, 10], BF16, tag="e")
        nc.scalar.activation(e[:], e_in[:], AF.Exp)

        sume = sbs.tile([128, NT, 2], F32, tag="sume")
        nc.vector.tensor_reduce(
            sume[:], e[:].rearrange("p t (s q) -> p t s q", s=2),
            axis=AX.X, op=ALU.add)
        sumr = sbs.tile([128, NT, 2], F32, tag="sumr")
        nc.vector.reciprocal(sumr[:], sume[:])
        # stil0 = mask/sume ; stil = (mask*inv)/sume  (U*inv shipped)
        stil0 = sbs.tile([128, NT, 2], F32, tag="stil0")
        nc.vector.tensor_tensor(out=stil0[:], in0=sumr[:], in1=U[:, b, :, :, 0],
                                op=ALU.mult)
        stil = sbs.tile([128, NT, 2], F32, tag="stil")
        nc.vector.tensor_tensor(out=stil[:], in0=sumr[:], in1=U[:, b, :, :, 1],
                                op=ALU.mult)

        # wt columns per side: [wg*inv x5 | invR | wg x5 | pad]
        wt = sbs.tile([128, NT, 24], BF16, tag="wt")
        wt4 = wt[:].rearrange("p t (s c) -> p t s c", s=2)
        e4 = e[:].rearrange("p t (s q) -> p t s q", s=2)
        nc.vector.tensor_tensor(out=wt4[:, :, :, 0:Q], in0=e4[:],
                                in1=stil[:].broadcast_to([128, NT, 2, Q]),
                                op=ALU.mult)
        nc.vector.tensor_tensor(out=wt4[:, :, :, 6:6 + Q], in0=e4[:],
                                in1=stil0[:].broadcast_to([128, NT, 2, Q]),
                                op=ALU.mult)
        nc.vector.tensor_copy(wt4[:, :, :, Q:6],
                              invR_ts.broadcast_to([128, NT, 2, 1]))
        st[b] = (ng, nl, wt, out_sb)

    def stage_pr(b):
        ng, nl, wt, out_sb = st[b]
        st[b] = None

        # P rows 0:11, R rows 32:43
        pr_ps = ps_pr.tile([48, C + 1], F32, tag="pr_ps")
        for t in range(NT):
            nc.tensor.matmul(pr_ps[0:11, :], wt[:, t, 0:11], ng[:, t, :],
                             start=(t == 0), stop=(t == NT - 1))
        for t in range(NT):
            nc.tensor.matmul(pr_ps[32:43, :], wt[:, t, 12:23], nl[:, t, :],
                             start=(t == 0), stop=(t == NT - 1),
                             tile_position=(0, 32))
        nc.scalar.copy(out_sb[0:11, 0:C + 1], pr_ps[0:11, :])
        nc.scalar.copy(out_sb[32:43, 0:C + 1], pr_ps[32:43, :])
        nc.gpsimd.dma_start(out[b], out_sb[:])

    for b in range(BL):
        stage_a(b)
    for b in range(BL):
        stage_tp(b)
    for b in range(BL):
        stage_pr(b)


def _neg_index():
    n2 = 2 * Q
    mask = np.ones((n2, n2), dtype=bool)
    np.fill_diagonal(mask, False)
    for i in range(Q):
        mask[i, Q + i] = False
        mask[Q + i, i] = False
    return np.stack([np.where(mask[r])[0] for r in range(n2)])


def _combine(results):
    T1 = 0.0
    G = 0.0
    alphas = []
    betas = []
    vs = []
    sims = []
    for r in results:
        o = np.asarray(r["out"], dtype=np.float64)  # [BL, 48, 400]
        P = o[:, 0:11, 0:C + 1]
        R = o[:, 32:43, 0:C + 1]
        sims.append(o[:, 0:10, 385:395])
        Pq, beta, sgc = P[:, 0:Q, 0:C], P[:, 6:6 + Q, C], P[:, Q, 0:C]
        Rq, v, slc = R[:, 0:Q, 0:C], R[:, 6:6 + Q, C], R[:, Q, 0:C]
        T1 += (Pq * Rq).sum()
        G += (sgc * slc).sum()
        alphas.append(np.einsum("bqc,bc->bq", Pq, slc))
        betas.append(beta)
        vs.append(v)
    alpha = np.concatenate(alphas, 0)
    beta = np.concatenate(betas, 0)
    v = np.concatenate(vs, 0)
    g = G / (B * N)
    T2 = ((alpha + (0.1 - g) * beta) * v).sum()
    loss1 = -0.15 * (T1 - T2)

    # query CE from raw gram matrices
    sim = np.concatenate(sims, 0)  # [B, 10, 10]
    d = np.einsum("bii->bi", sim)
    iq = 1.0 / np.maximum(np.sqrt(d), 1e-10)
    sh = sim * iq[:, :, None] * iq[:, None, :]
    rows = np.arange(2 * Q)
    pos = sh[:, rows, (rows + Q) % (2 * Q)]          # [B, 10]
    negs = sh[:, rows[:, None], _NEG_IDX]            # [B, 10, 8]
    logits = np.concatenate([pos[:, :, None], negs], axis=-1)
    m = logits.max(-1)
    ce = m + np.log(np.exp(logits - m[:, :, None]).sum(-1)) - pos
    loss2 = ce.mean()
    return np.float32(loss1 + loss2)


_NEG_IDX = _neg_index()


def _prep(gc, lc, q0, q1, att):
    """Build per-core input maps (host-side sharding + layout)."""
    # natural bf16 with ones column: [B, 128, 7, 385]
    def nat_pack(x):
        pad = np.zeros((B, NT * 128, C + 1), F8)
        pad[:, :N, :C] = x.astype(F8)
        pad[:, :, C] = 1.0
        return np.ascontiguousarray(
            pad.reshape(B, NT, 128, C + 1).transpose(0, 2, 1, 3))

    # transposed fp8: [B, 128, 3, 896]
    def tra_pack(x):
        t8 = np.zeros((B, C, NP), F8)
        t8[:, :, :N] = np.swapaxes(x, 1, 2).astype(F8)
        return np.ascontiguousarray(t8.reshape(B, NK, 128, NP).transpose(0, 2, 1, 3))

    z = np.concatenate([q0, q1], axis=1)  # [B, 10, 384]
    qn = np.linalg.norm(z, axis=-1)       # [B, 10]
    zhat8 = 8.0 * z / np.maximum(qn, 1e-10)[:, :, None]
    def z_pack(zv, dt):
        zt = np.zeros((B, C, 16), np.float32)
        zt[:, :, 0:10] = np.swapaxes(zv, 1, 2)
        return np.ascontiguousarray(
            zt.reshape(B, NK, 128, 16).transpose(0, 2, 1, 3).astype(dt))
    ztq_a = z_pack(zhat8, F8)
    ztb_a = z_pack(z, BF)

    natg_a = nat_pack(gc)
    natl_a = nat_pack(lc)
    trag_a = tra_pack(gc)
    tral_a = tra_pack(lc)

    # row inverse norms on host: inv [2, B, 7, 128] (s: 0=gc, 1=lc)
    nrm = np.stack([np.linalg.norm(gc, axis=-1), np.linalg.norm(lc, axis=-1)])
    invf = np.zeros((2, B, NT * 128), np.float32)
    invf[:, :, :N] = 1.0 / np.maximum(nrm, 1e-10)
    invf[:, :, N:] = 1e10
    invf = invf.reshape(2, B, NT, 128)
    mi = np.zeros((128, B, NT, 2, 3), np.float32)
    mi[:, :, :, :, 0] = invf.transpose(3, 1, 2, 0)
    mi[:, :, :, :, 1] = mi[:, :, :, :, 0] / 8.0
    mi[:, :, :, 0, 2] = mi[:, :, :, 0, 0]
    mi[:, :, :, 1, 2] = mi[:, :, :, 1, 0] / N

    S_sel = np.zeros((48, 16), BF)
    S_sel[np.arange(5), np.arange(5)] = 1
    S_sel[np.arange(32, 37), np.arange(5, 10)] = 1

    # mask U: [128, B, 7, 2]
    af = att.astype(np.float32)  # [128, 784]
    Uf = np.zeros((2, B, NT, 128), np.float32)
    Uf[0, :, :6, :] = af[:B, :768].reshape(B, 6, 128)
    Uf[0, :, 6, :16] = af[:B, 768:]
    Uf[1, :, :6, :] = af[B:, :768].reshape(B, 6, 128)
    Uf[1, :, 6, :16] = af[B:, 768:]
    Uf = Uf.transpose(3, 1, 2, 0)  # [128, B, 7, 2]
    U2 = np.stack([Uf, Uf * mi[:, :, :, :, 0]], axis=-1)  # [128, B, 7, 2, 2]

    in_maps = []
    for i in range(NCORES):
        s = slice(i * BL, (i + 1) * BL)
        in_maps.append({
            "natg": natg_a[s], "natl": natl_a[s],
            "trag": trag_a[s], "tral": tral_a[s],
            "ztq": ztq_a[s], "ztb": ztb_a[s],
            "misc": np.ascontiguousarray(mi[:, s]),
            "s_in": S_sel,
            "u_in": np.ascontiguousarray(U2[:, s]),
        })
    return in_maps


def kernel(all_queries_0, all_queries_1, gc_output, lc_output, attn_hard,
           gc_spatial_res=None, lc_spatial_res=None):
    if "nc" not in _CACHED:
        _CACHED["nc"] = _build()
    nc = _CACHED["nc"]

    gc = np.asarray(gc_output, dtype=np.float32)
    lc = np.asarray(lc_output, dtype=np.float32)[:, 0]
    q0 = np.asarray(all_queries_0, dtype=np.float32)
    q1 = np.asarray(all_queries_1, dtype=np.float32)
    att = np.asarray(attn_hard, dtype=np.int32).reshape(2 * B, N)

    in_maps = _prep(gc, lc, q0, q1, att)
    res = run_bass_kernel_spmd(nc, in_maps, core_ids=list(range(NCORES)))
    return _combine(res.results)


# revision 16
# speedup vs baseline: 1.3652x; 1.0027x over previous
"""Trainium2 Bass kernel for nn_AlignCriterion (align loss).

Data-parallel over batch: 8 batches per core, 8 cores. The O(B*N^2*C)
correlation/assignment einsums are algebraically collapsed (see _combine).

Layouts shipped from host per batch:
  natural  [128, 7, 385] bf16   x with a ones column  (P/R moving operand)
  transposed [128, 3, 896] fp8  x^T, n padded to 896  (asg moving operand)
  ztq      [128, 3, 16]   fp8   8 * normalized queries^T (asg stationary)
  ztb      [128, 3, 16]   bf16  raw queries^T (CE gram matrix)
  misc     [128, 7, 2, 3] f32   per-row 1/||x||: [inv, inv/8, invR]
  u        [128, 7, 2]    f32   attention masks (t, side)

Device per batch: sim = z z^T; asgT = ztq^T @ xT (both sides into one
PSUM tile, lc at rows 32:37 via tile_position); relu-drain to bf16;
7 combined PE transposes -> [128, 7, (2,5)]; masked softmax weights
wt = [wg*inv | invR | wg] (11 cols/side); P/R matmuls (R at rows 32:43).
The ones column gives beta/v; the inv column gives s_gc / s_lc/784.
Host combines partials in f64. Emission is software-pipelined: batch
b's transposes/PR are emitted after batch b+1's asg matmuls so the PE
stream never stalls on the DVE/ACT softmax round-trip.
"""

import sys

import numpy as np

sys.path.insert(0, "/opt/trn_rl_repo")

import ml_dtypes  # noqa: E402
import concourse.bass as bass  # noqa: E402,F401
import concourse.mybir as mybir  # noqa: E402
import concourse.tile as tile  # noqa: E402
from concourse import bacc  # noqa: E402
from concourse.bass_utils import run_bass_kernel_spmd  # noqa: E402
from concourse.masks import make_identity  # noqa: E402

F32 = mybir.dt.float32
BF16 = mybir.dt.bfloat16
FP8 = mybir.dt.float8e4
AF = mybir.ActivationFunctionType
ALU = mybir.AluOpType
AX = mybir.AxisListType

BF = ml_dtypes.bfloat16
F8 = ml_dtypes.float8_e4m3

B = 64
N = 784          # 28*28 spatial positions
C = 384
Q = 5
NCORES = 8
BL = B // NCORES  # batches per core = 8
NT = 7           # n tiles of 128
NK = 3           # c chunks of 128
NP = 896         # padded n for the transposed layout (7*128)
H = NP // 2      # psum half width (448)

_CACHED = {}


def _build():
    nc = bacc.Bacc("TRN2", target_bir_lowering=False, debug=False,
                   num_devices=NCORES)

    natm = nc.dram_tensor("natm", [BL, 128, 2, NT, C + 1], FP8, kind="ExternalInput").ap()
    tram = nc.dram_tensor("tram", [BL, 128, 2, NK, NP], FP8, kind="ExternalInput").ap()
    ztq = nc.dram_tensor("ztq", [BL, 128, NK, 16], FP8, kind="ExternalInput").ap()
    ztb = nc.dram_tensor("ztb", [BL, 128, NK, 16], BF16, kind="ExternalInput").ap()
    misc = nc.dram_tensor("misc", [128, BL, NT, 2, 3], F32, kind="ExternalInput").ap()
    s_in = nc.dram_tensor("s_in", [48, 16], BF16, kind="ExternalInput").ap()
    u_in = nc.dram_tensor("u_in", [128, BL, NT, 2, 2], F32, kind="ExternalInput").ap()
    out = nc.dram_tensor("out", [BL, 48, 400], F32, kind="ExternalOutput").ap()

    with tile.TileContext(nc) as tc:
        _kernel(tc, out, natm, tram, ztq, ztb, misc, u_in, s_in)

    # the installed walrus birverifier rejects EVENT_SEMAPHORE_RANGE_CLEAR
    # (opcode 176, emitted by the Tile kernel-tail sem cleanup). NRT re-inits
    # semaphores per execution, so drop the tail clear entirely.
    for fn in nc.m.functions:
        for blk in fn.blocks:
            il = blk.instructions
            for i in range(len(il) - 1, -1, -1):
                if isinstance(il[i], mybir.InstISA) and il[i].isa_opcode == 176:
                    del il[i]

    nc.compile()
    return nc


def _kernel(tc, out, natm, tram, ztq, ztb, misc, u_in, s_in):
    from contextlib import ExitStack
    with ExitStack() as ctx:
        _kernel_inner(ctx, tc, out, natm, tram, ztq, ztb, misc, u_in, s_in)


def _kernel_inner(ctx, tc, out, natm, tram, ztq, ztb, misc, u_in, s_in):
    nc = tc.nc

    consts = ctx.enter_context(tc.tile_pool(name="consts", bufs=1))
    sbin = ctx.enter_context(tc.tile_pool(name="sbin", bufs=3))
    sbnat = ctx.enter_context(tc.tile_pool(name="sbnat", bufs=8))
    sbq = ctx.enter_context(tc.tile_pool(name="sbq", bufs=3))
    sbs = ctx.enter_context(tc.tile_pool(name="sbs", bufs=4))
    sbo = ctx.enter_context(tc.tile_pool(name="sbo", bufs=4))
    ps_aa = ctx.enter_context(tc.tile_pool(name="ps_aa", bufs=2, space="PSUM"))
    ps_tp = ctx.enter_context(tc.tile_pool(name="ps_tp", bufs=2, space="PSUM"))
    ps_pr = ctx.enter_context(tc.tile_pool(name="ps_pr", bufs=2, space="PSUM"))

    S = consts.tile([48, 16], BF16, tag="S")
    nc.sync.dma_start(S[:], s_in[:, :])
    U = consts.tile([128, BL, NT, 2, 2], F32, tag="U")
    nc.sync.dma_start(U[:], u_in[:, :, :, :, :])
    MI = consts.tile([128, BL, NT, 2, 3], F32, tag="MI")
    nc.sync.dma_start(MI[:], misc[:, :, :, :, :])

    asgT_slots = []
    for j in range(BL):
        t = consts.tile([48, NP], BF16, tag=f"asgT{j}", name=f"asgT{j}")
        nc.gpsimd.memset(t[:], 0.0)
        asgT_slots.append(t)
    out_slots = []
    for j in range(BL):
        t = consts.tile([48, 400], F32, tag=f"outsb{j}", name=f"outsb{j}")
        nc.gpsimd.memset(t[:], 0.0)
        out_slots.append(t)

    st = [None] * BL  # per-batch live tiles for the lagged stage

    def stage_a(b):
        nat = sbnat.tile([128, 2, NT, C + 1], FP8, tag="nat")
        tra = sbin.tile([128, 2, NK, NP], FP8, tag="tra")
        nc.sync.dma_start(nat[:], natm[b])
        nc.sync.dma_start(tra[:], tram[b])
        ng, nl = nat[:, 0], nat[:, 1]
        tg, tl = tra[:, 0], tra[:, 1]
        zq = sbq.tile([128, NK, 16], FP8, tag="zq")
        zb = sbq.tile([128, NK, 16], BF16, tag="zb")
        nc.sync.dma_start(zq[:], ztq[b])
        nc.sync.dma_start(zb[:], ztb[b])

        out_sb = out_slots[b]
        asgT = asgT_slots[b]

        # sim (CE gram) + assignment logits share one 2-bank psum tile
        aa_ps = ps_aa.tile([48, 2, 512], F32, tag="aa_ps")
        for k in range(NK):
            nc.tensor.matmul(aa_ps[0:10, 0, 448:458], zb[:, k, 0:10],
                             zb[:, k, 0:10],
                             start=(k == 0), stop=(k == NK - 1))
        for h in range(2):
            for k in range(NK):
                nc.tensor.matmul(aa_ps[0:5, h, 0:H], zq[:, k, 0:Q],
                                 tg[:, k, H * h:H * (h + 1)],
                                 start=(k == 0), stop=(k == NK - 1))
        for h in range(2):
            for k in range(NK):
                nc.tensor.matmul(aa_ps[32:37, h, 0:H], zq[:, k, Q:10],
                                 tl[:, k, H * h:H * (h + 1)],
                                 start=(k == 0), stop=(k == NK - 1),
                                 tile_position=(0, 32))

        # relu + drain to bf16 (both on ACT: one queue, phase A has slack)
        nc.scalar.activation(
            asgT[0:5, 0:NP].rearrange("p (h n) -> p h n", h=2),
            aa_ps[0:5, :, 0:H], AF.Relu)
        nc.scalar.activation(
            asgT[32:37, 0:NP].rearrange("p (h n) -> p h n", h=2),
            aa_ps[32:37, :, 0:H], AF.Relu)
        nc.vector.tensor_copy(out_sb[0:10, 385:395], aa_ps[0:10, 0, 448:458])
        st[b] = (ng, nl, asgT, out_sb)

    def stage_tp(b):
        ng, nl, asgT, out_sb = st[b]

        # selector-"transpose" rows {0:5, 32:37} -> [128, 7, 10]:
        # plain matmul out = asgT.T @ S (stationary = asgT chunk)
        tp_ps = ps_tp.tile([128, NT, 10], F32, tag="tp_ps")
        for t in range(NT):
            nc.tensor.matmul(tp_ps[:, t, :],
                             asgT[0:37, 128 * t:128 * (t + 1)],
                             S[0:37, 0:10], start=True, stop=True)

        inv_ts = MI[:, b, :, :, 0]
        inv8_ts = MI[:, b, :, :, 1]
        invR_ts = MI[:, b, :, :, 2]

        # e = exp(asg * inv/8) ; tp cols: gc 0:5, lc 5:10
        e_in = sbs.tile([128, NT, 10], BF16, tag="e_in")
        nc.vector.tensor_tensor(
            out=e_in[:].rearrange("p t (s q) -> p t s q", s=2),
            in0=tp_ps[:].rearrange("p t (s q) -> p t s q", s=2),
            in1=inv8_ts.broadcast_to([128, NT, 2, Q]), op=ALU.mult)
        e = sbs.tile([128, NT, 10], BF16, tag="e")
        nc.scalar.activation(e[:], e_in[:], AF.Exp)

        sume = sbs.tile([128, NT, 2], F32, tag="sume")
        nc.vector.tensor_reduce(
            sume[:], e[:].rearrange("p t (s q) -> p t s q", s=2),
            axis=AX.X, op=ALU.add)
        sumr = sbs.tile([128, NT, 2], F32, tag="sumr")
        nc.vector.reciprocal(sumr[:], sume[:])
        # stil0 = mask/sume ; stil = (mask*inv)/sume  (U*inv shipped)
        stil0 = sbs.tile([128, NT, 2], F32, tag="stil0")
        nc.vector.tensor_tensor(out=stil0[:], in0=sumr[:], in1=U[:, b, :, :, 0],
                                op=ALU.mult)
        stil = sbs.tile([128, NT, 2], F32, tag="stil")
        nc.vector.tensor_tensor(out=stil[:], in0=sumr[:], in1=U[:, b, :, :, 1],
                                op=ALU.mult)

        # wt columns per side: [wg*inv x5 | invR | wg x5 | pad]
        wt = sbs.tile([128, NT, 24], BF16, tag="wt")
        wt4 = wt[:].rearrange("p t (s c) -> p t s c", s=2)
        e4 = e[:].rearrange("p t (s q) -> p t s q", s=2)
        nc.vector.tensor_tensor(out=wt4[:, :, :, 0:Q], in0=e4[:],
                                in1=stil[:].broadcast_to([128, NT, 2, Q]),
                                op=ALU.mult)
        nc.vector.tensor_tensor(out=wt4[:, :, :, 6:6 + Q], in0=e4[:],
                                in1=stil0[:].broadcast_to([128, NT, 2, Q]),
                                op=ALU.mult)
        nc.vector.tensor_copy(wt4[:, :, :, Q:6],
                              invR_ts.broadcast_to([128, NT, 2, 1]))
        st[b] = (ng, nl, wt, out_sb)

    def stage_pr(b):
        ng, nl, wt, out_sb = st[b]
        st[b] = None

        # P rows 0:11, R rows 32:43
        pr_ps = ps_pr.tile([48, C + 1], F32, tag="pr_ps")
        for t in range(NT):
            nc.tensor.matmul(pr_ps[0:11, :], wt[:, t, 0:11], ng[:, t, :],
                             start=(t == 0), stop=(t == NT - 1))
        for t in range(NT):
            nc.tensor.matmul(pr_ps[32:43, :], wt[:, t, 12:23], nl[:, t, :],
                             start=(t == 0), stop=(t == NT - 1),
                             tile_position=(0, 32))
        nc.scalar.copy(out_sb[0:11, 0:C + 1], pr_ps[0:11, :])
        nc.scalar.copy(out_sb[32:43, 0:C + 1], pr_ps[32:43, :])
        nc.gpsimd.dma_start(out[b], out_sb[:])

    for b in range(BL):
        stage_a(b)
    for b in range(BL):
        stage_tp(b)
    for b in range(BL):
        stage_pr(b)


def _neg_index():
    n2 = 2 * Q
    mask = np.ones((n2, n2), dtype=bool)
    np.fill_diagonal(mask, False)
    for i in range(Q):
        mask[i, Q + i] = False
        mask[Q + i, i] = False
    return np.stack([np.where(mask[r])[0] for r in range(n2)])


def _combine(results):
    T1 = 0.0
    G = 0.0
    alphas = []
    betas = []
    vs = []
    sims = []
    for r in results:
        o = np.asarray(r["out"], dtype=np.float64)  # [BL, 48, 400]
        P = o[:, 0:11, 0:C + 1]
        R = o[:, 32:43, 0:C + 1]
        sims.append(o[:, 0:10, 385:395])
        Pq, beta, sgc = P[:, 0:Q, 0:C], P[:, 6:6 + Q, C], P[:, Q, 0:C]
        Rq, v, slc = R[:, 0:Q, 0:C], R[:, 6:6 + Q, C], R[:, Q, 0:C]
        T1 += (Pq * Rq).sum()
        G += (sgc * slc).sum()
        alphas.append(np.einsum("bqc,bc->bq", Pq, slc))
        betas.append(beta)
        vs.append(v)
    alpha = np.concatenate(alphas, 0)
    beta = np.concatenate(betas, 0)
    v = np.concatenate(vs, 0)
    g = G / (B * N)
    T2 = ((alpha + (0.1 - g) * beta) * v).sum()
    loss1 = -0.15 * (T1 - T2)

    # query CE from raw gram matrices
    sim = np.concatenate(sims, 0)  # [B, 10, 10]
    d = np.einsum("bii->bi", sim)
    iq = 1.0 / np.maximum(np.sqrt(d), 1e-10)
    sh = sim * iq[:, :, None] * iq[:, None, :]
    rows = np.arange(2 * Q)
    pos = sh[:, rows, (rows + Q) % (2 * Q)]          # [B, 10]
    negs = sh[:, rows[:, None], _NEG_IDX]            # [B, 10, 8]
    logits = np.concatenate([pos[:, :, None], negs], axis=-1)
    m = logits.max(-1)
    ce = m + np.log(np.exp(logits - m[:, :, None]).sum(-1)) - pos
    loss2 = ce.mean()
    return np.float32(loss1 + loss2)


_NEG_IDX = _neg_index()


def _prep(gc, lc, q0, q1, att):
    """Build per-core input maps (host-side sharding + layout)."""
    # natural bf16 with ones column: [B, 128, 7, 385]
    def nat_pack(x):
        pad = np.zeros((B, NT * 128, C + 1), F8)
        pad[:, :N, :C] = x.astype(F8)
        pad[:, :, C] = 1.0
        return np.ascontiguousarray(
            pad.reshape(B, NT, 128, C + 1).transpose(0, 2, 1, 3))

    # transposed fp8: [B, 128, 3, 896]
    def tra_pack(x):
        t8 = np.zeros((B, C, NP), F8)
        t8[:, :, :N] = np.swapaxes(x, 1, 2).astype(F8)
        return np.ascontiguousarray(t8.reshape(B, NK, 128, NP).transpose(0, 2, 1, 3))

    z = np.concatenate([q0, q1], axis=1)  # [B, 10, 384]
    qn = np.linalg.norm(z, axis=-1)       # [B, 10]
    zhat8 = 8.0 * z / np.maximum(qn, 1e-10)[:, :, None]
    def z_pack(zv, dt):
        zt = np.zeros((B, C, 16), np.float32)
        zt[:, :, 0:10] = np.swapaxes(zv, 1, 2)
        return np.ascontiguousarray(
            zt.reshape(B, NK, 128, 16).transpose(0, 2, 1, 3).astype(dt))
    ztq_a = z_pack(zhat8, F8)
    ztb_a = z_pack(z, BF)

    natm_a = np.ascontiguousarray(
        np.stack([nat_pack(gc), nat_pack(lc)], axis=2))
    tram_a = np.ascontiguousarray(
        np.stack([tra_pack(gc), tra_pack(lc)], axis=2))

    # row inverse norms on host: inv [2, B, 7, 128] (s: 0=gc, 1=lc)
    nrm = np.stack([np.linalg.norm(gc, axis=-1), np.linalg.norm(lc, axis=-1)])
    invf = np.zeros((2, B, NT * 128), np.float32)
    invf[:, :, :N] = 1.0 / np.maximum(nrm, 1e-10)
    invf[:, :, N:] = 1e10
    invf = invf.reshape(2, B, NT, 128)
    mi = np.zeros((128, B, NT, 2, 3), np.float32)
    mi[:, :, :, :, 0] = invf.transpose(3, 1, 2, 0)
    mi[:, :, :, :, 1] = mi[:, :, :, :, 0] / 8.0
    mi[:, :, :, 0, 2] = mi[:, :, :, 0, 0]
    mi[:, :, :, 1, 2] = mi[:, :, :, 1, 0] / N

    S_sel = np.zeros((48, 16), BF)
    S_sel[np.arange(5), np.arange(5)] = 1
    S_sel[np.arange(32, 37), np.arange(5, 10)] = 1

    # mask U: [128, B, 7, 2]
    af = att.astype(np.float32)  # [128, 784]
    Uf = np.zeros((2, B, NT, 128), np.float32)
    Uf[0, :, :6, :] = af[:B, :768].reshape(B, 6, 128)
    Uf[0, :, 6, :16] = af[:B, 768:]
    Uf[1, :, :6, :] = af[B:, :768].reshape(B, 6, 128)
    Uf[1, :, 6, :16] = af[B:, 768:]
    Uf = Uf.transpose(3, 1, 2, 0)  # [128, B, 7, 2]
    U2 = np.stack([Uf, Uf * mi[:, :, :, :, 0]], axis=-1)  # [128, B, 7, 2, 2]

    in_maps = []
    for i in range(NCORES):
        s = slice(i * BL, (i + 1) * BL)
        in_maps.append({
            "natm": natm_a[s], "tram": tram_a[s],
            "ztq": ztq_a[s], "ztb": ztb_a[s],
            "misc": np.ascontiguousarray(mi[:, s]),
            "s_in": S_sel,
            "u_in": np.ascontiguousarray(U2[:, s]),
        })
    return in_maps


def kernel(all_queries_0, all_queries_1, gc_output, lc_output, attn_hard,
           gc_spatial_res=None, lc_spatial_res=None):
    if "nc" not in _CACHED:
        _CACHED["nc"] = _build()
    nc = _CACHED["nc"]

    gc = np.asarray(gc_output, dtype=np.float32)
    lc = np.asarray(lc_output, dtype=np.float32)[:, 0]
    q0 = np.asarray(all_queries_0, dtype=np.float32)
    q1 = np.asarray(all_queries_1, dtype=np.float32)
    att = np.asarray(attn_hard, dtype=np.int32).reshape(2 * B, N)

    in_maps = _prep(gc, lc, q0, q1, att)
    res = run_bass_kernel_spmd(nc, in_maps, core_ids=list(range(NCORES)))
    return _combine(res.results)


# revision 20
# speedup vs baseline: 1.4800x; 1.0841x over previous
"""Trainium2 Bass kernel for nn_AlignCriterion (align loss).

Data-parallel over batch: 8 batches per core, 8 cores. The O(B*N^2*C)
correlation/assignment einsums are algebraically collapsed (see _combine).

Layouts shipped from host per batch:
  natural  [128, 7, 385] bf16   x with a ones column  (P/R moving operand)
  transposed [128, 3, 896] fp8  x^T, n padded to 896  (asg moving operand)
  ztq      [128, 3, 16]   fp8   8 * normalized queries^T (asg stationary)
  ztb      [128, 3, 16]   bf16  raw queries^T (CE gram matrix)
  misc     [128, 7, 2, 3] f32   per-row 1/||x||: [inv, inv/8, invR]
  u        [128, 7, 2]    f32   attention masks (t, side)

Device per batch: sim = z z^T; asgT = ztq^T @ xT (both sides into one
PSUM tile, lc at rows 32:37 via tile_position); relu-drain to bf16;
7 combined PE transposes -> [128, 7, (2,5)]; masked softmax weights
wt = [wg*inv | invR | wg] (11 cols/side); P/R matmuls (R at rows 32:43).
The ones column gives beta/v; the inv column gives s_gc / s_lc/784.
Host combines partials in f64. Emission is software-pipelined: batch
b's transposes/PR are emitted after batch b+1's asg matmuls so the PE
stream never stalls on the DVE/ACT softmax round-trip.
"""

import sys

import numpy as np

sys.path.insert(0, "/opt/trn_rl_repo")

import ml_dtypes  # noqa: E402
import concourse.bass as bass  # noqa: E402,F401
import concourse.mybir as mybir  # noqa: E402
import concourse.tile as tile  # noqa: E402
from concourse import bacc  # noqa: E402
from concourse.bass_utils import run_bass_kernel_spmd  # noqa: E402
from concourse.masks import make_identity  # noqa: E402

F32 = mybir.dt.float32
BF16 = mybir.dt.bfloat16
FP8 = mybir.dt.float8e4
AF = mybir.ActivationFunctionType
ALU = mybir.AluOpType
AX = mybir.AxisListType

BF = ml_dtypes.bfloat16
F8 = ml_dtypes.float8_e4m3

B = 64
N = 784          # 28*28 spatial positions
C = 384
Q = 5
NCORES = 8
BL = B // NCORES  # batches per core = 8
NT = 7           # n tiles of 128
NK = 3           # c chunks of 128
NP = 896         # padded n for the transposed layout (7*128)
H = NP // 2      # psum half width (448)

_CACHED = {}


def _build():
    nc = bacc.Bacc("TRN2", target_bir_lowering=False, debug=False,
                   num_devices=NCORES)

    natm = nc.dram_tensor("natm", [BL, 128, 2, NT, 400], FP8, kind="ExternalInput").ap()
    tram = nc.dram_tensor("tram", [BL, 128, 2, NK, NP], FP8, kind="ExternalInput").ap()
    ztq = nc.dram_tensor("ztq", [BL, 128, NK, 16], FP8, kind="ExternalInput").ap()
    ztb = nc.dram_tensor("ztb", [BL, 128, NK, 16], BF16, kind="ExternalInput").ap()
    misc = nc.dram_tensor("misc", [128, BL, NT, 2, 3], F32, kind="ExternalInput").ap()
    s_in = nc.dram_tensor("s_in", [48, 16], BF16, kind="ExternalInput").ap()
    u_in = nc.dram_tensor("u_in", [128, BL, NT, 2, 2], F32, kind="ExternalInput").ap()
    out = nc.dram_tensor("out", [BL, 48, 416], F32, kind="ExternalOutput").ap()

    with tile.TileContext(nc) as tc:
        _kernel(tc, out, natm, tram, ztq, ztb, misc, u_in, s_in)

    # the installed walrus birverifier rejects EVENT_SEMAPHORE_RANGE_CLEAR
    # (opcode 176, emitted by the Tile kernel-tail sem cleanup). NRT re-inits
    # semaphores per execution, so drop the tail clear entirely.
    for fn in nc.m.functions:
        for blk in fn.blocks:
            il = blk.instructions
            for i in range(len(il) - 1, -1, -1):
                if isinstance(il[i], mybir.InstISA) and il[i].isa_opcode == 176:
                    del il[i]

    nc.compile()
    return nc


def _kernel(tc, out, natm, tram, ztq, ztb, misc, u_in, s_in):
    from contextlib import ExitStack
    with ExitStack() as ctx:
        _kernel_inner(ctx, tc, out, natm, tram, ztq, ztb, misc, u_in, s_in)


def _kernel_inner(ctx, tc, out, natm, tram, ztq, ztb, misc, u_in, s_in):
    nc = tc.nc

    consts = ctx.enter_context(tc.tile_pool(name="consts", bufs=1))
    sbin = ctx.enter_context(tc.tile_pool(name="sbin", bufs=3))
    sbnat = ctx.enter_context(tc.tile_pool(name="sbnat", bufs=8))
    sbq = ctx.enter_context(tc.tile_pool(name="sbq", bufs=3))
    sbs = ctx.enter_context(tc.tile_pool(name="sbs", bufs=4))
    sbo = ctx.enter_context(tc.tile_pool(name="sbo", bufs=4))
    ps_aa = ctx.enter_context(tc.tile_pool(name="ps_aa", bufs=2, space="PSUM"))
    ps_tp = ctx.enter_context(tc.tile_pool(name="ps_tp", bufs=2, space="PSUM"))
    ps_pr = ctx.enter_context(tc.tile_pool(name="ps_pr", bufs=2, space="PSUM"))

    S = consts.tile([48, 16], BF16, tag="S")
    nc.sync.dma_start(S[:], s_in[:, :])
    U = consts.tile([128, BL, NT, 2, 2], F32, tag="U")
    nc.sync.dma_start(U[:], u_in[:, :, :, :, :])
    MI = consts.tile([128, BL, NT, 2, 3], F32, tag="MI")
    nc.sync.dma_start(MI[:], misc[:, :, :, :, :])

    asgT_slots = []
    for j in range(BL):
        t = consts.tile([48, NP], BF16, tag=f"asgT{j}", name=f"asgT{j}")
        nc.gpsimd.memset(t[:], 0.0)
        asgT_slots.append(t)
    out_slots = []
    for j in range(BL):
        t = consts.tile([48, 416], F32, tag=f"outsb{j}", name=f"outsb{j}")
        nc.gpsimd.memset(t[:], 0.0)
        out_slots.append(t)

    st = [None] * BL  # per-batch live tiles for the lagged stage

    ld = [None] * BL

    def stage_load_a(b):
        tra = sbin.tile([128, 2, NK, NP], FP8, tag="tra")
        nc.sync.dma_start(tra[:], tram[b])
        zq = sbq.tile([128, NK, 16], FP8, tag="zq")
        zb = sbq.tile([128, NK, 16], BF16, tag="zb")
        nc.sync.dma_start(zq[:], ztq[b])
        nc.sync.dma_start(zb[:], ztb[b])
        ld[b] = (tra, zq, zb)

    def stage_load_nat(b):
        nat = sbnat.tile([128, 2, NT, 400], FP8, tag="nat")
        nc.sync.dma_start(nat[:], natm[b])
        tra, zq, zb = ld[b]
        ld[b] = (tra, zq, zb, nat)

    def stage_a(b):
        tra, zq, zb, nat = ld[b]
        ld[b] = None
        ng, nl = nat[:, 0], nat[:, 1]
        tg, tl = tra[:, 0], tra[:, 1]

        out_sb = out_slots[b]
        asgT = asgT_slots[b]

        # sim (CE gram) + assignment logits share one 2-bank psum tile
        aa_ps = ps_aa.tile([48, 2, 512], F32, tag="aa_ps")
        for k in range(NK):
            nc.tensor.matmul(aa_ps[0:10, 0, 448:458], zb[:, k, 0:10],
                             zb[:, k, 0:10],
                             start=(k == 0), stop=(k == NK - 1))
        DR = mybir.MatmulPerfMode.DoubleRow
        for h in range(2):
            nc.tensor.matmul(aa_ps[0:5, h, 0:H], zq[:, 0:2, 0:Q],
                             tg[:, 0:2, H * h:H * (h + 1)],
                             start=True, stop=False, perf_mode=DR)
            nc.tensor.matmul(aa_ps[0:5, h, 0:H], zq[:, 2, 0:Q],
                             tg[:, 2, H * h:H * (h + 1)],
                             start=False, stop=True)
        for h in range(2):
            for k in range(NK):
                nc.tensor.matmul(aa_ps[32:37, h, 0:H], zq[:, k, Q:10],
                                 tl[:, k, H * h:H * (h + 1)],
                                 start=(k == 0), stop=(k == NK - 1),
                                 tile_position=(0, 32))

        # relu + drain to bf16 (both on ACT: one queue, phase A has slack)
        nc.scalar.activation(
            asgT[0:5, 0:NP].rearrange("p (h n) -> p h n", h=2),
            aa_ps[0:5, :, 0:H], AF.Relu)
        nc.scalar.activation(
            asgT[32:37, 0:NP].rearrange("p (h n) -> p h n", h=2),
            aa_ps[32:37, :, 0:H], AF.Relu)
        nc.vector.tensor_copy(out_sb[0:10, 400:410], aa_ps[0:10, 0, 448:458])
        st[b] = (ng, nl, asgT, out_sb)

    def stage_tp(b):
        ng, nl, asgT, out_sb = st[b]

        # selector-"transpose" rows {0:5, 32:37} -> [128, 7, 10]:
        # plain matmul out = asgT.T @ S (stationary = asgT chunk)
        tp_ps = ps_tp.tile([128, NT, 10], F32, tag="tp_ps")
        for t in range(NT):
            nc.tensor.matmul(tp_ps[:, t, :],
                             asgT[0:37, 128 * t:128 * (t + 1)],
                             S[0:37, 0:10], start=True, stop=True)

        inv_ts = MI[:, b, :, :, 0]
        inv8_ts = MI[:, b, :, :, 1]
        invR_ts = MI[:, b, :, :, 2]

        # e = exp(asg * inv/8) ; tp cols: gc 0:5, lc 5:10
        e_in = sbs.tile([128, NT, 10], BF16, tag="e_in")
        nc.vector.tensor_tensor(
            out=e_in[:].rearrange("p t (s q) -> p t s q", s=2),
            in0=tp_ps[:].rearrange("p t (s q) -> p t s q", s=2),
            in1=inv8_ts.broadcast_to([128, NT, 2, Q]), op=ALU.mult)
        e = sbs.tile([128, NT, 10], BF16, tag="e")
        nc.scalar.activation(e[:], e_in[:], AF.Exp)

        sume = sbs.tile([128, NT, 2], F32, tag="sume")
        nc.vector.tensor_reduce(
            sume[:], e[:].rearrange("p t (s q) -> p t s q", s=2),
            axis=AX.X, op=ALU.add)
        sumr = sbs.tile([128, NT, 2], F32, tag="sumr")
        nc.vector.reciprocal(sumr[:], sume[:])
        # stil0 = mask/sume ; stil = (mask*inv)/sume  (U*inv shipped)
        stil0 = sbs.tile([128, NT, 2], F32, tag="stil0")
        nc.vector.tensor_tensor(out=stil0[:], in0=sumr[:], in1=U[:, b, :, :, 0],
                                op=ALU.mult)
        stil = sbs.tile([128, NT, 2], F32, tag="stil")
        nc.vector.tensor_tensor(out=stil[:], in0=sumr[:], in1=U[:, b, :, :, 1],
                                op=ALU.mult)

        # wt columns per side: [wg*inv x5 | invR | wg x5 | pad] (x16)
        wt = sbs.tile([128, NT, 32], BF16, tag="wt")
        wt4 = wt[:].rearrange("p t (s c) -> p t s c", s=2)
        e4 = e[:].rearrange("p t (s q) -> p t s q", s=2)
        nc.vector.tensor_tensor(out=wt4[:, :, :, 0:Q], in0=e4[:],
                                in1=stil[:].broadcast_to([128, NT, 2, Q]),
                                op=ALU.mult)
        nc.vector.tensor_tensor(out=wt4[:, :, :, 6:6 + Q], in0=e4[:],
                                in1=stil0[:].broadcast_to([128, NT, 2, Q]),
                                op=ALU.mult)
        nc.vector.tensor_copy(wt4[:, :, :, Q:6],
                              invR_ts.broadcast_to([128, NT, 2, 1]))
        st[b] = (ng, nl, wt, out_sb)

    def stage_pr(b):
        ng, nl, wt, out_sb = st[b]
        st[b] = None

        # P rows 0:11, R rows 32:43
        pr_ps = ps_pr.tile([48, 400], F32, tag="pr_ps")
        for t in range(NT):
            nc.tensor.matmul(pr_ps[0:11, :], wt[:, t, 0:11], ng[:, t, :],
                             start=(t == 0), stop=(t == NT - 1))
        for t in range(NT):
            nc.tensor.matmul(pr_ps[32:43, :], wt[:, t, 16:27], nl[:, t, :],
                             start=(t == 0), stop=(t == NT - 1),
                             tile_position=(0, 32))
        nc.scalar.copy(out_sb[0:11, 0:400], pr_ps[0:11, :])
        nc.scalar.copy(out_sb[32:43, 0:400], pr_ps[32:43, :])
        nc.gpsimd.dma_start(out[b], out_sb[:])

    for b in range(BL):
        stage_load_a(b)
    for b in range(BL):
        stage_load_nat(b)
    for b in range(BL):
        stage_a(b)
    for b in range(BL):
        stage_tp(b)
    for b in range(BL):
        stage_pr(b)


def _neg_index():
    n2 = 2 * Q
    mask = np.ones((n2, n2), dtype=bool)
    np.fill_diagonal(mask, False)
    for i in range(Q):
        mask[i, Q + i] = False
        mask[Q + i, i] = False
    return np.stack([np.where(mask[r])[0] for r in range(n2)])


def _combine(results):
    T1 = 0.0
    G = 0.0
    alphas = []
    betas = []
    vs = []
    sims = []
    for r in results:
        o = np.asarray(r["out"], dtype=np.float64)  # [BL, 48, 416]
        P = o[:, 0:11, 0:C + 1] / 16.0
        R = o[:, 32:43, 0:C + 1] / 16.0
        sims.append(o[:, 0:10, 400:410])
        Pq, beta, sgc = P[:, 0:Q, 0:C], P[:, 6:6 + Q, C], P[:, Q, 0:C]
        Rq, v, slc = R[:, 0:Q, 0:C], R[:, 6:6 + Q, C], R[:, Q, 0:C]
        T1 += (Pq * Rq).sum()
        G += (sgc * slc).sum()
        alphas.append(np.einsum("bqc,bc->bq", Pq, slc))
        betas.append(beta)
        vs.append(v)
    alpha = np.concatenate(alphas, 0)
    beta = np.concatenate(betas, 0)
    v = np.concatenate(vs, 0)
    g = G / (B * N)
    T2 = ((alpha + (0.1 - g) * beta) * v).sum()
    loss1 = -0.15 * (T1 - T2)

    # query CE from raw gram matrices
    sim = np.concatenate(sims, 0)  # [B, 10, 10]
    d = np.einsum("bii->bi", sim)
    iq = 1.0 / np.maximum(np.sqrt(d), 1e-10)
    sh = sim * iq[:, :, None] * iq[:, None, :]
    rows = np.arange(2 * Q)
    pos = sh[:, rows, (rows + Q) % (2 * Q)]          # [B, 10]
    negs = sh[:, rows[:, None], _NEG_IDX]            # [B, 10, 8]
    logits = np.concatenate([pos[:, :, None], negs], axis=-1)
    m = logits.max(-1)
    ce = m + np.log(np.exp(logits - m[:, :, None]).sum(-1)) - pos
    loss2 = ce.mean()
    return np.float32(loss1 + loss2)


_NEG_IDX = _neg_index()


def _prep(gc, lc, q0, q1, att):
    """Build per-core input maps (host-side sharding + layout)."""
    # natural bf16 with ones column: [B, 128, 7, 385]
    def nat_pack(x):
        pad = np.zeros((B, NT * 128, 400), F8)
        pad[:, :N, :C] = x.astype(F8)
        pad[:, :, C] = 1.0
        return np.ascontiguousarray(
            pad.reshape(B, NT, 128, 400).transpose(0, 2, 1, 3))

    # transposed fp8: [B, 128, 3, 896]
    def tra_pack(x):
        t8 = np.zeros((B, C, NP), F8)
        t8[:, :, :N] = np.swapaxes(x, 1, 2).astype(F8)
        return np.ascontiguousarray(t8.reshape(B, NK, 128, NP).transpose(0, 2, 1, 3))

    z = np.concatenate([q0, q1], axis=1)  # [B, 10, 384]
    qn = np.linalg.norm(z, axis=-1)       # [B, 10]
    zhat8 = 8.0 * z / np.maximum(qn, 1e-10)[:, :, None]
    def z_pack(zv, dt):
        zt = np.zeros((B, C, 16), np.float32)
        zt[:, :, 0:10] = np.swapaxes(zv, 1, 2)
        return np.ascontiguousarray(
            zt.reshape(B, NK, 128, 16).transpose(0, 2, 1, 3).astype(dt))
    ztq_a = z_pack(zhat8, F8)
    ztb_a = z_pack(z, BF)

    natm_a = np.ascontiguousarray(
        np.stack([nat_pack(gc), nat_pack(lc)], axis=2))
    tram_a = np.ascontiguousarray(
        np.stack([tra_pack(gc), tra_pack(lc)], axis=2))

    # row inverse norms on host: inv [2, B, 7, 128] (s: 0=gc, 1=lc)
    nrm = np.stack([np.linalg.norm(gc, axis=-1), np.linalg.norm(lc, axis=-1)])
    invf = np.zeros((2, B, NT * 128), np.float32)
    invf[:, :, :N] = 1.0 / np.maximum(nrm, 1e-10)
    invf = invf.reshape(2, B, NT, 128)
    mi = np.zeros((128, B, NT, 2, 3), np.float32)
    mi[:, :, :, :, 0] = invf.transpose(3, 1, 2, 0)
    mi[:, :, :, :, 1] = mi[:, :, :, :, 0] / 8.0
    mi[:, :, :, 0, 2] = mi[:, :, :, 0, 0] * 16.0
    mi[:, :, :, 1, 2] = mi[:, :, :, 1, 0] * (16.0 / N)

    S_sel = np.zeros((48, 16), BF)
    S_sel[np.arange(5), np.arange(5)] = 1
    S_sel[np.arange(32, 37), np.arange(5, 10)] = 1

    # mask U: [128, B, 7, 2]
    af = att.astype(np.float32)  # [128, 784]
    Uf = np.zeros((2, B, NT, 128), np.float32)
    Uf[0, :, :6, :] = af[:B, :768].reshape(B, 6, 128)
    Uf[0, :, 6, :16] = af[:B, 768:]
    Uf[1, :, :6, :] = af[B:, :768].reshape(B, 6, 128)
    Uf[1, :, 6, :16] = af[B:, 768:]
    Uf = Uf.transpose(3, 1, 2, 0)  # [128, B, 7, 2]
    U2 = np.stack([Uf * 16.0, 16.0 * Uf * mi[:, :, :, :, 0]], axis=-1)

    in_maps = []
    for i in range(NCORES):
        s = slice(i * BL, (i + 1) * BL)
        in_maps.append({
            "natm": natm_a[s], "tram": tram_a[s],
            "ztq": ztq_a[s], "ztb": ztb_a[s],
            "misc": np.ascontiguousarray(mi[:, s]),
            "s_in": S_sel,
            "u_in": np.ascontiguousarray(U2[:, s]),
        })
    return in_maps


def kernel(all_queries_0, all_queries_1, gc_output, lc_output, attn_hard,
           gc_spatial_res=None, lc_spatial_res=None):
    if "nc" not in _CACHED:
        _CACHED["nc"] = _build()
    nc = _CACHED["nc"]

    gc = np.asarray(gc_output, dtype=np.float32)
    lc = np.asarray(lc_output, dtype=np.float32)[:, 0]
    q0 = np.asarray(all_queries_0, dtype=np.float32)
    q1 = np.asarray(all_queries_1, dtype=np.float32)
    att = np.asarray(attn_hard, dtype=np.int32).reshape(2 * B, N)

    in_maps = _prep(gc, lc, q0, q1, att)
    res = run_bass_kernel_spmd(nc, in_maps, core_ids=list(range(NCORES)))
    return _combine(res.results)
